# revision 101
# baseline (speedup 1.0000x reference)
"""AdaptiveSparsityAttention TRN2 kernel (8 NeuronCores, SPMD data-parallel).

Problem (B=2, S=1024, D=512, H=2 heads, dh=256, hidden=128):
  q,k,v = x@Wq, x@Wk, x@Wv (split 2 heads); scores = q@k^T/16
  a_i = q_mean@W1[:dh]+b1, c_j = k_mean@W1[dh:]
  z[i,j] = W2 . relu(a_i + c_j)          (sigmoid(z+b2)>0.5  <=>  z > -b2)
  attn = softmax(mask(scores));  out = (attn@v)@Wo + bo

Sharding: 8 cores = 2 batches x 4 query-chunks of 256 rows. Each core
computes its output chunk fully locally, no collectives.

Precision structure (measured):
  - z dtype ladder (measured L2 vs ref): fp32 4.87e-3, fp16 T+wsel
    1.5701e-2 (PASSES the 2e-2 gate with 27% margin, deterministic),
    bf16 5.8e-2 (FAILS). Current kernel uses fp16 T tiles + fp16 wsel
    (z matmuls 1 cyc/row vs fp32's 4 + no 110ns fp32 LDWs; -4us HW).
    NOTE: with fp16 T the DVE producer must emit full relu(ct+a) (2-op
    max,add); the 1-op max(ct,-a) + alpha-threshold-fold trick (exact
    in fp32, machinery still present via selv/thr2, selv currently all
    zero) makes |T| larger and would roughly double the fp16 rounding
    hit. If more margin is ever needed, revert to fp32 T + wsel_s and
    re-enable the alpha fold: that measured 178.9->183.5us @ 4.87e-3.
  - a/c computed exactly via 3 bf16 hi/lo streams (Mhi@xhi + Mhi@xlo +
    Mlo@xhi, residual 2^-18): kills the 2MB fp32 xT DMA (bf16 xlo
    instead) and runs on full-width bf16 MMs.
  - Everything downstream of the mask is bf16 (4.1e-3 contribution).

Performance structure (per core; HW best 170719ns at full clock
(run-to-run band 170.7-172.7us; includes a 12-MM PE warm-up on cf
data -- the first a/c MMs otherwise start at the cold p-state, 634
vs 216ns; extending to 34 warm-ups overshot xb-arrival and measured
SLOWER, 174.1 -- and the v-projection + restructured ti-split tail
with av-ti0 copies on ACT) vs 197472 session-start baseline; the
chip alternates into a 2.0GHz P0 throttle under sustained benching --
all durations scale ~1.2x; check zMM med (379 full clock for fp16 z)
before comparing runs. Late additions: tp(1,0) in the mid (tail PE
otherwise waits ~5us, head-of-line), bf16 output tile+DMA, weight
DMAs held behind an at_s-dependent gpsimd dummy op (their packets
otherwise steal half the 16 shared DMA engines from the front-
critical x/mq transfers), blk1 tpat 78:50. Also measured: gpsimd
tensor_scalar is 15284ns per [128,1024] tile (13x ACT -- useless as
a third T-producer, though port contention with DVE was NOT seen);
emitting v_us before the tail softmaxes (or even in the mid) is
~neutral: the Tile scheduler reorders ops by its own cost model --
the trace shows the v MMs placed after blk1's z regardless of
emission position, always opening ~at em(0,1)'s final scale. The
~4us tail-PE gap is scheduler-chosen, not a data dep; attacking it
needs Tile-level control, not emission order):
  - z stream: 2 blocks x 128 steps of [T produce on DVE (~806ns 2-op)
    or ACT (~1136ns relu)] + 2 fp16 wsel-delta matmuls. Producer-bound
    ~500ns/step at 10:6 DVE:ACT (length-128 tpat). T pool 36 bufs
    (fp16 tiles are 2KB/part).
  - DVE op costs are (N/Accel + ~208)/0.96 cyc; ACT is (N+352)/1.2,
    dtype-independent. PSUM reads are 1-port (no 2x). These floors
    bind the producers; gpsimd shares the DVE SBUF port (untested).
  - DMA: full-tensor / dt-half transfers only (4-8KB per-partition
    contiguous packets). Sliced 512B-1KB-run transfers halve queue
    rate (descriptor-issue bound). gpsimd SWDGE queue measured
    ~170-190GB/s with 4KB packets and carries all late-needed weights.
    ~6us fixed framework preamble before any DMA data moves.
  - Schedule: [stage A: a/c bf16 streams, jc0 then a then jc1]
    [blk0 z] [mid: mask0, sm(0,0), sm(1,0), tp(0,0) (copies on ACT),
    24 preTs -- preTs BEFORE sm would stall the PE mid work (pool
    knot); sm(0,0) waits ~8-10us on the exp<-sc<-qk chain regardless,
    preT(16) ahead of it fills the V hole] [blk1 z] [tail: mask1,
    sm(0,1), sm(1,1), tp(1,0) copies first on ACT (ahead of the 8 v
    copies -- HOL), v proj, tp(.,1) copies on V, av and out-proj split
    by ti with per-half output DMAs]. All PSUM->SBUF copies in
    mid/tail ride ACT except tp(.,1)+av-ti1 on V; any V-queue op in
    the mid delays blk1's T-production 1:1.
  - Transposes batch 4 per PSUM tile -> one [P,4,128] copy (fewer,
    fatter copies; no 2-buf transpose/copy ping-pong).
  - Measured SLOWER and reverted (this session): qk+sc+exp pre-blk0
    (193-201us: pool runway < PE setup chunk, producers stall; ALSO
    retried at pool=36 with fp16 z: 184.9 vs 178.9), v/av/oproj in the
    mid (mid V/ACT bloat delays blk1 1:1), f32r z matmuls (ISA: f32r
    weights <=64 array cols, dst quadrant fixed 0/64 -- partitions
    96-127 unreachable, delta trick impossible; also BIR requires
    producers to round-to-f32r).
  - Engine totals are near-balanced (PE ~135, DVE ~130, ACT ~115 of
    ~179us span): further wins need WORK reduction, not scheduling.
"""

import sys

if "/opt/trn_rl_repo" not in sys.path:
    sys.path.insert(0, "/opt/trn_rl_repo")

import numpy as np
import ml_dtypes

import concourse.bass as bass  # noqa: F401
import concourse.tile as tile
from concourse import bacc, mybir
from concourse.bass_utils import run_bass_kernel_spmd

F32 = mybir.dt.float32
FP16 = mybir.dt.float16
BF16 = mybir.dt.bfloat16
AL = mybir.AluOpType
AF = mybir.ActivationFunctionType

B, S, D = 2, 1024, 512
DH = D // 2          # 256 per-head dim
HID = 128            # predictor hidden
NCHUNK = S // 4      # 256 query rows per core
P = 128

def _mkpat(*ratios):
    """length-128 producer pattern from per-16 (v,s) ratios, cycled."""
    base = {
        (10, 6): ["v", "s", "v", "s", "v", "v", "s", "v",
                  "s", "v", "v", "s", "v", "v", "s", "v"],
        (9, 7): ["v", "s", "v", "s", "v", "v", "s", "v",
                 "s", "v", "v", "s", "v", "s", "v", "s"],
    }
    out = []
    for g in range(8):
        out += base[ratios[g % len(ratios)]]
    return out


# knobs (test.py may override before first kernel() call)
CONFIG = {
    "trace": False,
    "tmpdir": None,
    # per-row producer pattern (length 128, indexed by i%128): 'v'=DVE
    # (806ns 2-op), 's'=ACT (1136ns relu). With fp16 z the PE is no
    # longer co-bound and DVE (T + mask/sm + tail copies, ~150us busy)
    # binds: 76:52 equalizes DVE/ACT totals (~137 each).
    "tpat": _mkpat((10, 6)),
    "tpat1": _mkpat((10, 6), (9, 7)),
    "t_bufs": 36,
}

_STATE = {}


def _emit(tc, nc, t):
    sl512 = [slice(0, 512), slice(512, 1024)]

    with tc.tile_pool(name="big", bufs=1) as big:
        # ---- persistent residents ----
        cf_s = big.tile([P, 68], F32, name="cf_s")             # b1|thr|wsel32|selv
        b1_s = cf_s[:, 0:1]
        thr_s = cf_s[:, 1:2]
        wsel_s = cf_s[:, 2:66]
        selv_s = cf_s[:, 66:68]   # 1.0 where block b's row i is DVE-produced
        thr2_s = big.tile([P, 2], F32, name="thr2_s")  # per-row mask threshold
        wself_s = big.tile([P, 64], FP16, name="wself_s")  # fp16 wsel window
        cb_s = big.tile([1, D + P], BF16, name="cb_s")          # bo | ones
        bo_s = cb_s[:, 0:D]
        one_s = cb_s[:, D : D + P]
        ident = big.tile([P, P], BF16, name="ident")

        at_s = big.tile([P, NCHUNK], F32, name="at_s")    # a^T + b1, [h, i]
        nat_s = big.tile([P, NCHUNK], F32, name="nat_s")  # -(a^T + b1)
        ct_s = big.tile([P, S], F32, name="ct_s")          # c^T, [h, j]

        xbt_s = big.tile([P, 4, S], BF16, name="xbt_s")       # x^T bf16
        wqb_s = big.tile([P, 4, D], BF16, name="wqb_s")       # Wq/16 bf16
        wkb_s = big.tile([P, 4, D], BF16, name="wkb_s")
        wvb_s = big.tile([P, 4, D], BF16, name="wvb_s")
        wob_s = big.tile([P, 4, D], BF16, name="wob_s")
        qt_s = big.tile([P, 4, NCHUNK], BF16, name="qt_s")  # q^T/16 [dout, i]
        kt_s = big.tile([P, 4, S], BF16, name="kt_s")       # k^T [dout, j]
        v_s = big.tile([P, 8, D], BF16, name="v_s")         # v [j(8 tiles), d]
        otr_s = big.tile([P, 4, NCHUNK], BF16, name="otr_s")  # out^T [d, i]

        with (
            tc.tile_pool(name="pjp", bufs=1, space="PSUM") as pjp,
            tc.tile_pool(name="zps", bufs=1, space="PSUM") as zpsp,
        ):
            # -------- stage A: DMAs + exact a/c (bf16 hi/lo splits) --------
            with tc.tile_pool(name="stageA", bufs=1) as sa:
                xlo_s = sa.tile([P, 4, S], BF16, name="xlo_s")   # x - bf16(x)
                mqh_s = sa.tile([P, 4, 2 * HID], BF16, name="mqh_s")
                mql_s = sa.tile([P, 4, 2 * HID], BF16, name="mql_s")
                # a/c are computed exactly via 3 bf16 streams
                # (Mhi@xhi + Mhi@xlo + Mlo@xhi; the dropped Mlo@xlo term is
                # ~2^-18 relative). Front-load x_hi/x_lo/M on the 2 HW
                # queues; weights ride gpsimd SWDGE (needed only in mid).
                # dt-half transfers keep 4KB/partition contiguous packets
                # (full queue rate) while letting the a/c matmuls start on
                # the first half. xb alone on scalar (feeds 2 of 3 streams)
                nc.scalar.dma_start(xbt_s[:, 0:2, :], t["xbT"][:, 0:2, :])
                nc.scalar.dma_start(xbt_s[:, 2:4, :], t["xbT"][:, 2:4, :])
                nc.sync.dma_start(cf_s[:], t["constsf"])
                nc.sync.dma_start(mqh_s[:], t["mqh"])
                nc.sync.dma_start(mql_s[:], t["mql"])
                nc.sync.dma_start(xlo_s[:, 0:2, :], t["xlo"][:, 0:2, :])
                nc.sync.dma_start(xlo_s[:, 2:4, :], t["xlo"][:, 2:4, :])


                # exact a/c: xb-based streams first (xb lands first), the
                # xlo stream last; ct j-half 0 first, then a, then j-half 1
                # so at/nat + the jc0 copy land while jc1 still computes.
                a_streams = [(mqh_s, xbt_s), (mql_s, xbt_s), (mqh_s, xlo_s)]
                ct_pss = [
                    pjp.tile([P, 512], F32, tag="vps", bufs=2, name="ct_ps")
                    for _ in range(2)
                ]

                def ct_mms(hf, streams, start=False, stop=False):
                    ops = [(m_, x_, dt_) for m_, x_ in streams for dt_ in range(4)]
                    for n_, (m_, x_, dt_) in enumerate(ops):
                        nc.tensor.matmul(
                            ct_pss[hf][:], m_[:, dt_, HID : 2 * HID],
                            x_[:, dt_, sl512[hf]],
                            start=(start and n_ == 0),
                            stop=(stop and n_ == len(ops) - 1),
                        )

                at_ps = zpsp.tile([P, NCHUNK], F32, tag="z", bufs=2, name="at_ps")
                # PE warm-up: the first real a/c MMs otherwise run at the
                # cold p-state (~634ns vs ~216 warm for [128,512] bf16).
                # Throwaway MMs on the first-arriving cf data ramp the HAM
                # while x/mq land; every one uses start=True so the real
                # chain's own reset wins.
                for _ in range(12):
                    nc.tensor.matmul(
                        at_ps[0:64, 0:64], wsel_s[:, 0:64], wsel_s[:, 0:64],
                        start=True, stop=True,
                    )
                ct_mms(0, a_streams[0:2], start=True)          # xb-based, jc0
                for st, (m_, x_) in enumerate(a_streams):      # a (all dt)
                    for dt_ in range(4):
                        nc.tensor.matmul(
                            at_ps[:], m_[:, dt_, 0:HID], x_[:, dt_, 0:NCHUNK],
                            start=(st == 0 and dt_ == 0),
                            stop=(st == 2 and dt_ == 3),
                        )
                ct_mms(0, a_streams[2:3], stop=True)           # xlo, jc0
                nc.vector.tensor_scalar(at_s[:], at_ps[:], b1_s[:], None, AL.add)
                # weights are needed only from the mid (~90us) but their
                # packets share the 16 DMA engines with the front-critical
                # x/mq transfers. A dummy gpsimd op depending on at_s holds
                # the SWDGE queue until the critical transfers have landed.
                dly_s = big.tile([P, 1], F32, name="dly_s")
                nc.gpsimd.tensor_copy(dly_s[:], at_s[:, 0:1])
                nc.gpsimd.dma_start(wqb_s[:], t["wq"])
                nc.gpsimd.dma_start(wkb_s[:], t["wk"])
                nc.gpsimd.dma_start(cb_s[:], t["constsb"])
                nc.gpsimd.dma_start(ident[:], t["identb"])
                nc.gpsimd.dma_start(wvb_s[:], t["wv"])
                nc.gpsimd.dma_start(wob_s[:], t["wo"])
                nc.vector.tensor_copy(wself_s[:], wsel_s[:])
                nc.scalar.copy(ct_s[:, sl512[0]], ct_pss[0][:])
                ct_mms(1, a_streams, start=True, stop=True)    # jc1 (all)
                nc.vector.tensor_scalar(nat_s[:], at_s[:], -1.0, None, AL.mult)

                # alpha_b[i] = sum_h W2[h]*at[h, 128b+i]: DVE 'v' steps emit
                # T' = max(ct, -at) (1-op), so their z rows miss sum W2*at.
                # Fold into the mask threshold: thr2 = thr - alpha*selv.
                for b_ in range(2):
                    al_ps = zpsp.tile([P, 1], F32, tag="z", bufs=2, name="al_ps")
                    nc.tensor.matmul(
                        al_ps[:], at_s[:, P * b_ : P * (b_ + 1)],
                        wsel_s[:, 32:33], start=True, stop=True,
                    )
                    nc.vector.scalar_tensor_tensor(
                        thr2_s[:, b_ : b_ + 1], al_ps[:], -1.0,
                        selv_s[:, b_ : b_ + 1], AL.mult, AL.mult,
                    )
                    nc.vector.tensor_scalar(
                        thr2_s[:, b_ : b_ + 1], thr2_s[:, b_ : b_ + 1],
                        thr_s[:], None, AL.add,
                    )

                nc.scalar.copy(ct_s[:, sl512[1]], ct_pss[1][:])

            # ---------------- z blocks / mid / tail ------------------------
            with (
                tc.tile_pool(name="Tp", bufs=CONFIG["t_bufs"]) as Tp,
                tc.tile_pool(name="work", bufs=2) as work,
            ):
                att_sb = [
                    work.tile([P, 8, NCHUNK], BF16, tag="attnT", bufs=2,
                              name=f"a_sb{h}")
                    for h in range(2)
                ]
                # exp(scores) for all 4 (head, ti) pairs: mask-independent,
                # computed in the mid gap; masked in softmax_finish later.
                e_sb = [
                    [work.tile([P, S], BF16, tag="e", bufs=4, name=f"e{h}_{ti}")
                     for ti in range(2)]
                    for h in range(2)
                ]

                def make_T(ii):
                    # fp16 T: the z matmuls run at 1 cyc/row with cheap
                    # weight loads (vs fp32's 4 passes + 110ns LDWs). relu
                    # form (not the max/alpha fold): relu's smaller T
                    # magnitudes halve the fp16 rounding hit on z.
                    T = Tp.tile([P, S], FP16, tag="T", name=f"T{ii}")
                    pat = CONFIG["tpat"] if ii < 128 else CONFIG["tpat1"]
                    if pat[ii % 128] == "v":
                        nc.vector.tensor_scalar(
                            T[:], ct_s[:], nat_s[:, ii : ii + 1],
                            at_s[:, ii : ii + 1], AL.max, AL.add,
                        )
                    else:
                        nc.scalar.activation(
                            T[:], ct_s[:], AF.Relu, bias=at_s[:, ii : ii + 1]
                        )
                    return T

                def emit_zblock(blk, pre=()):
                    # z accumulation for 128 query rows; col groups MUST cycle
                    # (g=step%4): serial same-group fp32 mms run at 2cyc/col,
                    # cycling pipelines passes across the 4 array quadrants.
                    # Keep this stream PURE: a full-array matmul inserted here
                    # drains the quadrant pipeline (~1us each).
                    # `pre` carries T tiles already emitted during the mid gap.
                    zp = zpsp.tile([P, S], F32, tag="z", bufs=2, name=f"zp{blk}")
                    for step in range(128):
                        k, g = step // 4, step % 4
                        i = 32 * g + k
                        ii = blk * 128 + i
                        T = pre[step] if step < len(pre) else make_T(ii)
                        for jc in range(2):
                            nc.tensor.matmul(
                                zp[32 * g : 32 * g + 32, sl512[jc]],
                                wself_s[:, 32 - k : 64 - k],
                                T[:, sl512[jc]],
                                start=(k == 0), stop=(k == 31),
                                tile_position=(0, 32 * g),
                                skip_group_check=True,
                            )
                    return zp

                def qkv_units():
                    # projections: dense bf16 PE chains (FWL weight loads),
                    # decomposed into units so blk1 T-producers interleave
                    us = []
                    def qt_u(dout):
                        def go():
                            qt_ps = pjp.tile([P, NCHUNK], F32, tag="vps", bufs=2, name="qt_ps")
                            for dt_ in range(4):
                                nc.tensor.matmul(
                                    qt_ps[:], wqb_s[:, dt_, 128 * dout : 128 * (dout + 1)],
                                    xbt_s[:, dt_, 0:NCHUNK], start=(dt_ == 0), stop=(dt_ == 3),
                                )
                            nc.scalar.copy(qt_s[:, dout, :], qt_ps[:])
                        return go
                    def kt_u(dout, jc):
                        def go():
                            kt_ps = pjp.tile([P, 512], F32, tag="vps", bufs=2, name="kt_ps")
                            for dt_ in range(4):
                                nc.tensor.matmul(
                                    kt_ps[:],
                                    wkb_s[:, dt_, 128 * dout : 128 * (dout + 1)],
                                    xbt_s[:, dt_, sl512[jc]],
                                    start=(dt_ == 0), stop=(dt_ == 3),
                                )
                            nc.scalar.copy(kt_s[:, dout, sl512[jc]], kt_ps[:])
                        return go
                    def v_u(jt):
                        def go():
                            v_ps = pjp.tile([P, D], F32, tag="vps", bufs=2, name="v_ps")
                            for dt_ in range(4):
                                nc.tensor.matmul(
                                    v_ps[:], xbt_s[:, dt_, 128 * jt : 128 * (jt + 1)],
                                    wvb_s[:, dt_, :], start=(dt_ == 0), stop=(dt_ == 3),
                                )
                            nc.scalar.copy(v_s[:, jt, :], v_ps[:])
                        return go
                    us += [qt_u(d) for d in range(4)]
                    us += [kt_u(d, jc) for d in range(4) for jc in range(2)]
                    return us, [v_u(jt) for jt in range(8)]

                def emit_scores(h, ti):
                    # scores + exp for rows [128*ti,128*(ti+1)) of head h.
                    # Mask-independent: runs in the mid gap for both ti.
                    for jc in range(2):
                        scp = pjp.tile([P, 512], F32, tag="vps", bufs=2,
                                       name=f"sc{h}{ti}{jc}")
                        for dt_ in range(2):
                            nc.tensor.matmul(
                                scp[:],
                                qt_s[:, 2 * h + dt_, 128 * ti : 128 * (ti + 1)],
                                kt_s[:, 2 * h + dt_, sl512[jc]],
                                start=(dt_ == 0), stop=(dt_ == 1),
                            )
                        # scores/16 bounded (|sc|<~7): exp cannot overflow,
                        # rowmax subtraction dropped (identical result)
                        nc.scalar.activation(
                            e_sb[h][ti][:, sl512[jc]], scp[:], AF.Exp
                        )

                em_sb = [[None, None], [None, None]]

                mask_sb = [None, None]

                def emit_mask(blk, zp):
                    m = work.tile([P, S], BF16, tag="mask", bufs=2, name=f"mask{blk}")
                    mask_sb[blk] = m
                    nc.vector.tensor_scalar(
                        m[:], zp[:], thr2_s[:, blk : blk + 1], None, AL.is_gt
                    )

                def softmax_finish(h, ti, zp):
                    # em = (e*mask + ind)/(sum + 1024*ind)
                    e = e_sb[h][ti]
                    em = work.tile([P, S], BF16, tag="em", bufs=4, name=f"em{h}{ti}")
                    em_sb[h][ti] = em
                    ssum = work.tile([P, 1], F32, tag="ssum", name="ssum")
                    nc.vector.scalar_tensor_tensor(
                        em[:], e[:], 0.0, mask_sb[ti][:], AL.add, AL.mult,
                        accum_out=ssum[:],
                    )
                    # fully-masked rows: reference = uniform 1/1024.
                    ind = work.tile([P, 1], F32, tag="ind", name="ind")
                    nc.vector.tensor_scalar(ind[:], ssum[:], 0.0, None, AL.is_equal)
                    s2 = work.tile([P, 1], F32, tag="s2", name="s2")
                    nc.vector.tensor_scalar(s2[:], ind[:], 1024.0, ssum[:], AL.mult, AL.add)
                    rinv = work.tile([P, 1], F32, tag="rinv", name="rinv")
                    nc.vector.reciprocal(rinv[:], s2[:])
                    nc.vector.tensor_scalar(em[:], em[:], ind[:], rinv[:], AL.add, AL.mult)

                def emit_transposes(h, ti, eng="s"):
                    # mid copies on ACT (the DVE queue is the blk1
                    # pace-setter); tail copies on V (ACT carries the v
                    # copies there). 4 transposes batch into one PSUM tile
                    # -> one [P,4,128] copy (fewer, fatter copies; no
                    # transpose/copy ping-pong on the 2 bufs).
                    em = em_sb[h][ti]
                    for g in range(2):
                        tp_ps = pjp.tile([P, 4, P], BF16, tag="tp", bufs=2, name="tp_ps")
                        for j2 in range(4):
                            jt = 4 * g + j2
                            nc.tensor.transpose(
                                tp_ps[:, j2, :], em[:, 128 * jt : 128 * (jt + 1)],
                                ident[:],
                            )
                        dst = att_sb[h][:, 4 * g : 4 * (g + 1), 128 * ti : 128 * (ti + 1)]
                        if eng == "s":
                            nc.scalar.copy(dst, tp_ps[:])
                        else:
                            nc.vector.tensor_copy(dst, tp_ps[:])

                def emit_av(h, ti):
                    # ti-split: the ti=0 half depends only on mid-gap
                    # transposes, so it runs in the mid; tail does ti=1 only
                    isl = slice(128 * ti, 128 * (ti + 1))
                    for dt_ in range(2):
                        ot_ps = pjp.tile([P, P], F32, tag="vps", bufs=2, name="ot_ps")
                        for jt in range(8):
                            nc.tensor.matmul(
                                ot_ps[:],
                                v_s[:, jt, 256 * h + 128 * dt_ : 256 * h + 128 * (dt_ + 1)],
                                att_sb[h][:, jt, isl],
                                start=(jt == 0), stop=(jt == 7),
                            )
                        # ti=0 copies on ACT (V carries the critical tail
                        # softmax/tp chain); ti=1 split V/ACT
                        if ti == 1 and dt_ % 2 == 0:
                            nc.vector.tensor_copy(otr_s[:, 2 * h + dt_, isl], ot_ps[:])
                        else:
                            nc.scalar.copy(otr_s[:, 2 * h + dt_, isl], ot_ps[:])

                def emit_oproj(ti):
                    # out rows [128ti,128ti+128): bias + 4 otr-weighted MMs,
                    # then SBUF copy + the output DMA for that half
                    o_ps = zpsp.tile([P, D], F32, tag="z", bufs=2, name=f"o_ps{ti}")
                    nc.tensor.matmul(o_ps[:], one_s[:], bo_s[:], start=True, stop=False)
                    for dt_ in range(4):
                        nc.tensor.matmul(
                            o_ps[:], otr_s[:, dt_, 128 * ti : 128 * (ti + 1)],
                            wob_s[:, dt_, :], start=False, stop=(dt_ == 3),
                        )
                    o_sb = work.tile([P, D], BF16, tag="osb", bufs=2, name="o_sb")
                    if ti == 0:
                        nc.scalar.copy(o_sb[:], o_ps[:])
                    else:
                        nc.vector.tensor_copy(o_sb[:], o_ps[:])
                    nc.sync.dma_start(t["out"][128 * ti : 128 * (ti + 1), :], o_sb[:])

                # ---- emission schedule ----
                zp0 = emit_zblock(0)

                # mid gap: projections/scores/exp + blk0 softmax, with blk1
                # T-producers interleaved. The V queue carries ONLY mask/
                # softmax/preT (all PSUM copies go to ACT): any other V op
                # delays blk1's T-production 1:1.
                pre = []
                def preT(n):
                    for _ in range(n):
                        step = len(pre)
                        k, g = step // 4, step % 4
                        pre.append(make_T(128 + 32 * g + k))
                qk_us, v_us = qkv_units()
                for i_, u in enumerate((qk_us[0], qk_us[1], qk_us[4], qk_us[5], qk_us[6], qk_us[7])):
                    u()
                    if i_ % 2:
                        preT(1)
                emit_mask(0, zp0)
                emit_scores(0, 0)
                emit_scores(0, 1)
                # sm(0,0) blocks on exp<-sc<-qk for ~10us after mask0; preTs
                # emitted ahead of it fill that V hole (pool slots free at
                # blk0's drain rate, ~2.3/us)
                preT(16)
                softmax_finish(0, 0, zp0)
                emit_transposes(0, 0, "s")
                for i_, u in enumerate((qk_us[2], qk_us[3], qk_us[8], qk_us[9], qk_us[10], qk_us[11])):
                    u()
                    if i_ % 2:
                        preT(1)
                emit_scores(1, 0)
                emit_scores(1, 1)
                softmax_finish(1, 0, zp0)
                emit_transposes(1, 0, "s")
                # V projection in the mid: its copies ride ACT (hidden
                # under DVE's longer blk1 share) and the PE here is idle;
                # the tail's first PE ops (av ti=0) are then dep-free at
                # zp1-end instead of stalling ~5us behind the v chain.
                for u in v_us:
                    u()
                preT(8)

                zp1 = emit_zblock(1, pre)

                # tail: V runs mask1 + both ti=1 softmaxes then the tp/av
                # copies; PE opens dep-free with the ti=0 attention half
                # (v/tp(.,0) all mid-complete) while the sm chains run.
                emit_mask(1, zp1)
                softmax_finish(0, 1, zp1)
                softmax_finish(1, 1, zp1)
                emit_av(0, 0)
                emit_av(1, 0)
                emit_oproj(0)
                emit_transposes(0, 1, "v")
                emit_transposes(1, 1, "v")
                emit_av(0, 1)
                emit_av(1, 1)
                emit_oproj(1)


def _build():
    if "nc" in _STATE:
        return _STATE["nc"]
    nc = bacc.Bacc(
        "TRN2", target_bir_lowering=False, debug=False, enable_asserts=True,
        num_devices=8,
    )
    t = {}
    t["xbT"] = nc.dram_tensor("xbT", [P, 4, S], BF16, kind="ExternalInput").ap()
    t["xlo"] = nc.dram_tensor("xlo", [P, 4, S], BF16, kind="ExternalInput").ap()
    t["wq"] = nc.dram_tensor("wq", [P, 4, D], BF16, kind="ExternalInput").ap()
    t["wk"] = nc.dram_tensor("wk", [P, 4, D], BF16, kind="ExternalInput").ap()
    t["wv"] = nc.dram_tensor("wv", [P, 4, D], BF16, kind="ExternalInput").ap()
    t["wo"] = nc.dram_tensor("wo", [P, 4, D], BF16, kind="ExternalInput").ap()
    t["mqh"] = nc.dram_tensor("mqh", [P, 4, 2 * HID], BF16, kind="ExternalInput").ap()
    t["mql"] = nc.dram_tensor("mql", [P, 4, 2 * HID], BF16, kind="ExternalInput").ap()
    t["constsf"] = nc.dram_tensor("constsf", [P, 68], F32, kind="ExternalInput").ap()
    t["constsb"] = nc.dram_tensor("constsb", [1, D + P], BF16, kind="ExternalInput").ap()
    t["identb"] = nc.dram_tensor("identb", [P, P], BF16, kind="ExternalInput").ap()
    t["out"] = nc.dram_tensor("out", [NCHUNK, D], BF16, kind="ExternalOutput").ap()

    with tile.TileContext(nc) as tc:
        _emit(tc, nc, t)
    nc.compile()
    _STATE["nc"] = nc
    return nc


def _prep_in_maps(inputs):
    bf16 = ml_dtypes.bfloat16
    x = np.ascontiguousarray(np.asarray(inputs["x"], np.float32))
    Wq = np.asarray(inputs["Wq"], np.float32)
    Wk = np.asarray(inputs["Wk"], np.float32)
    Wv = np.asarray(inputs["Wv"], np.float32)
    Wo = np.asarray(inputs["Wo"], np.float32)
    bo = np.asarray(inputs["bo"], np.float32)
    W1 = np.asarray(inputs["W1"], np.float64)
    b1 = np.asarray(inputs["b1"], np.float32)
    W2 = np.asarray(inputs["W2"], np.float32)
    b2 = np.asarray(inputs["b2"], np.float32)

    wq_m = 0.5 * (Wq[:, :DH].astype(np.float64) + Wq[:, DH:].astype(np.float64))
    wk_m = 0.5 * (Wk[:, :DH].astype(np.float64) + Wk[:, DH:].astype(np.float64))
    Mq = np.ascontiguousarray((wq_m @ W1[:DH]).astype(np.float32))
    Mk = np.ascontiguousarray((wk_m @ W1[DH:]).astype(np.float32))

    def chunk(a):
        # [D, N] -> [P, 4, N]: partition-chunked layout for one-shot DMA
        return np.ascontiguousarray(a.reshape(4, P, -1).transpose(1, 0, 2))

    constsf = np.zeros((P, 68), np.float32)
    constsf[:, 0] = b1
    constsf[:, 1] = -float(b2[0])
    constsf[:, 2 + 32] = W2[:, 0]          # wsel32 window buffer
    # selv stays 0: both producer paths emit full relu(ct+a) T tiles, so
    # no alpha threshold correction is needed (thr2 == thr)
    constsb = np.zeros((1, D + P), bf16)
    constsb[0, :D] = bo.astype(bf16)
    constsb[0, D:] = np.ones(P, bf16)

    M = np.concatenate([Mq, Mk], axis=1)
    Mh = M.astype(bf16)
    shared = dict(
        wq=chunk((Wq / 16.0).astype(bf16)),
        wk=chunk(Wk.astype(bf16)),
        wv=chunk(Wv.astype(bf16)),
        wo=chunk(Wo.astype(bf16)),
        mqh=chunk(Mh),
        mql=chunk((M - Mh.astype(np.float32)).astype(bf16)),
        constsf=constsf, constsb=constsb,
        identb=np.eye(P, dtype=bf16),
    )
    in_maps = []
    for c in range(8):
        b, i0 = c // 4, (c % 4) * NCHUNK
        m = dict(shared)
        # roll x columns so this core's query chunk sits at j=0..255; the
        # softmax result is invariant to a consistent j-permutation of
        # keys/values/mask, and it lets `a` start from the first x DMA.
        xr = np.roll(x[b].T, -i0, axis=1)
        xh = xr.astype(bf16)
        m["xbT"] = chunk(xh)
        m["xlo"] = chunk((xr - xh.astype(np.float32)).astype(bf16))
        in_maps.append(m)
    return in_maps


def kernel(**inputs):
    nc = _build()
    in_maps = _prep_in_maps(inputs)
    res = run_bass_kernel_spmd(
        nc, in_maps, core_ids=list(range(8)),
        trace=CONFIG["trace"], tmpdir=CONFIG["tmpdir"],
    )
    _STATE["last_result"] = res
    out = np.empty((B, S, D), np.float32)
    for c in range(8):
        b, i0 = c // 4, (c % 4) * NCHUNK
        out[b, i0 : i0 + NCHUNK] = res.results[c]["out"].astype(np.float32)
    return out

